# revision 3
# baseline (speedup 1.0000x reference)
"""Trainium2 Bass kernel for nn_AttentionBlock (B=32, C=1024, H=W=32, nh=1).

Reference computation (per batch b, with S = H*W = 1024):
    qkv = w_qkv @ x_b            # [3C, S], 1x1 conv == channel matmul
    q, k, v = split(qkv)
    logits[t,s] = (q[:,t] . k[:,s]) / sqrt(C)
    attn = softmax_s(logits)
    h[t,s] = attn[t,s] * sum_c v[c,s]
    out = w_proj @ h + b_proj + x_b

Algebraic simplifications (all weight-only, precomputed on host):
  * logits = x^T (M x) with M = Wq^T Wk  -> q/k never materialized.
  * sum_c v[c,s] = (sum_c Wv[c,:]) . x[:,s] = vs, computed on the
    vector/scalar engines + one ones-matmul (partition reduction).
  * h = attn .* (1 vs^T).

Precision plan (measured: fp8 DoubleRow = 1.87x bf16 matmul throughput):
  * stage A (y = M x): fp8e4 DoubleRow pairs, M and x quantized on host
    (clip to +-240 for TRN e4m3). Error here is softened by the softmax
    normalization downstream.
  * stage B (logits = x^T y): f32r x f32r (11+ mantissa bits) - the
    softmax input stays near-exact.
  * stage C (out = w_proj @ h): bf16 x bf16.
  * residual adds the resident f32 x exactly.
  Simulated end-to-end rel err ~9e-3 on both observed input draws
  (gate 2e-2).

Sharding: data-parallel over batch, 4 batches per core on 8 cores.
"""

import os
import sys

import numpy as np

for _p in ("/opt/trn_rl_repo", "/opt/pypackages"):
    if _p not in sys.path:
        sys.path.insert(0, _p)

import ml_dtypes

import concourse.bass as bass
import concourse.tile as tile
from concourse import bacc, mybir
from concourse.bass_utils import run_bass_kernel_spmd
from concourse.tile_rust import add_dep_helper

B, C, HH, WW = 32, 1024, 32, 32
S = HH * WW          # 1024 spatial positions
P = 128              # partitions
KC = C // P          # 8 chunks along channel dim
TC = S // P          # 8 chunks along spatial (t) dim
NN = 512             # matmul moving free dim
NCH = S // NN        # 2 free-dim halves
N_CORES = 8
BPC = B // N_CORES   # batches per core
SCALE = 1.0 / np.sqrt(float(C))  # folded into the exp

f32 = mybir.dt.float32
f32r = mybir.dt.float32r
bf16 = mybir.dt.bfloat16
fp8 = mybir.dt.float8e4
DR = mybir.MatmulPerfMode.DoubleRow


def build_nc(bpc: int = BPC):
    nc = bacc.Bacc(
        "TRN2",
        target_bir_lowering=False,
        debug=False,
        enable_asserts=False,
    )

    # x pre-rounded to f32r on host (stage-B stationary, vs input, and
    # the residual via bitcast back to f32 — costs ~2^-12 rounding)
    x_d = nc.dram_tensor("x", [bpc, C, S], f32r, kind="ExternalInput")
    # x pre-quantized to fp8 on host, layout [p, k, s] with c = k*128+p
    x8_d = nc.dram_tensor("x8", [bpc, P, KC, S], fp8, kind="ExternalInput")
    # weight stripes pre-arranged on host: [chunk, c, 128]
    mt_d = nc.dram_tensor("mt", [KC, C, P], fp8, kind="ExternalInput")
    wpt_d = nc.dram_tensor("wpt", [KC, C, P], bf16, kind="ExternalInput")
    wvs_d = nc.dram_tensor("wvs", [C], f32, kind="ExternalInput")
    ones_d = nc.dram_tensor("ones", [P, P], f32r, kind="ExternalInput")
    bp_d = nc.dram_tensor("bp", [C], f32, kind="ExternalInput")
    out_d = nc.dram_tensor("out", [bpc, C, S], f32, kind="ExternalOutput")

    with tile.TileContext(nc) as tc:
        with (
            tc.tile_pool(name="weights", bufs=1) as wpool,
            tc.tile_pool(name="xc", bufs=11) as xpool,
            tc.tile_pool(name="x8", bufs=2) as x8pool,
            tc.tile_pool(name="y", bufs=1) as ypool,
            tc.tile_pool(name="h", bufs=1) as hpool,
            tc.tile_pool(name="vsb", bufs=2) as vpool,
            tc.tile_pool(name="vacc", bufs=2) as vaccpool,
            tc.tile_pool(name="vtmp", bufs=4) as vtmppool,
            tc.tile_pool(name="osb", bufs=4) as opool,
            tc.tile_pool(name="small", bufs=8) as spool,
            tc.tile_pool(name="psA", bufs=3, space="PSUM") as psA,
            tc.tile_pool(name="psB", bufs=2, space="PSUM") as psB,
            tc.tile_pool(name="psC", bufs=3, space="PSUM") as psC,
        ):
            # ---- small resident weights first (cheap DMAs) ----
            wvs_sb = wpool.tile([P, KC], f32, tag="wvs")
            nc.sync.dma_start(wvs_sb[:], wvs_d.rearrange("(ko ki) -> ki ko", ki=P))
            bp_sb = wpool.tile([P, KC], f32, tag="bp")
            nc.sync.dma_start(bp_sb[:], bp_d.rearrange("(o p) -> p o", p=P))
            ones_sb = wpool.tile([P, P], f32r, tag="ones")
            nc.sync.dma_start(ones_sb[:], ones_d[:, :])
            # warm the PE clock (HAM) with throwaway matmuls while the
            # first batch's weights/x DMAs are in flight
            wu = psA.tile([P, NN], f32, tag="psA")
            for _ in range(25):
                nc.tensor.matmul(
                    wu[:, 0:64], ones_sb[:], ones_sb[:, 0:64],
                    start=True, stop=True,
                )
            wpt_sb = wpool.tile([P, TC, C], bf16, tag="wpt")
            mt_sb = wpool.tile([P, KC, C], fp8, tag="mt")

            for b in range(bpc):
                # ---- load x: fp8 (stage A) first, f32 chunks after ----
                x8_sb = x8pool.tile([P, KC, S], fp8, tag="x8")
                xc = []
                for k in range(KC):
                    t = xpool.tile([P, S], f32r, tag="xc")
                    xc.append(t)
                if b == 0:
                    # Critical startup set: all mt stripes (1 MiB fp8) + x8
                    # (1 MiB) — the stage-A inputs. Everything else chains
                    # behind them so concurrent DMA queues don't dilute the
                    # bandwidth the first matmuls are waiting on.
                    crit = [nc.sync.dma_start(x8_sb[:], x8_d[b])]
                    for mc in range(KC):
                        crit.append(
                            nc.sync.dma_start(
                                mt_sb[:, :, mc * P : (mc + 1) * P],
                                mt_d[mc].rearrange("(ko ki) m -> ki ko m", ki=P),
                            )
                        )
                    gate = crit[-1].ins
                    noncrit = []
                    for k in range(KC):
                        noncrit.append(
                            nc.sync.dma_start(xc[k][:], x_d[b, k * P : (k + 1) * P, :])
                        )
                    for inst in noncrit:
                        add_dep_helper(
                            inst.ins, gate, sync=True,
                            reason="startup: critical DMAs first",
                        )
                else:
                    nc.sync.dma_start(x8_sb[:], x8_d[b])
                    for k in range(KC):
                        nc.sync.dma_start(xc[k][:], x_d[b, k * P : (k + 1) * P, :])

                # ---- stage A2a: vacc[p,s] = sum_k wvs[k*128+p] * x[k][p,s]
                # products on ACT; pairwise-tree adds split over DVE and
                # GPSIMD so neither serializes. PE only does the final
                # 128-partition ones-matmul.
                vacc = vaccpool.tile([P, S], f32r, tag="vacc")

                def _vprod(k):
                    vt = vtmppool.tile([P, S], f32, tag="vtmp")
                    nc.scalar.activation(
                        vt[:], xc[k][:],
                        mybir.ActivationFunctionType.Copy,
                        scale=wvs_sb[:, k : k + 1],
                    )
                    return vt

                p0, p1 = _vprod(0), _vprod(1)
                nc.vector.tensor_tensor(p0[:], p0[:], p1[:], mybir.AluOpType.add)
                p2, p3 = _vprod(2), _vprod(3)
                nc.gpsimd.tensor_tensor(p2[:], p2[:], p3[:], mybir.AluOpType.add)
                nc.gpsimd.tensor_tensor(p0[:], p0[:], p2[:], mybir.AluOpType.add)
                p4, p5 = _vprod(4), _vprod(5)
                nc.vector.tensor_tensor(p4[:], p4[:], p5[:], mybir.AluOpType.add)
                p6, p7 = _vprod(6), _vprod(7)
                nc.gpsimd.tensor_tensor(p6[:], p6[:], p7[:], mybir.AluOpType.add)
                nc.vector.tensor_tensor(p4[:], p4[:], p6[:], mybir.AluOpType.add)
                nc.vector.tensor_tensor(vacc[:], p0[:], p4[:], mybir.AluOpType.add)

                # ---- stage A: y = M x via fp8 DoubleRow pairs ----
                y_sb = ypool.tile([P, KC, S], f32r, tag="y")
                for mc in range(KC):
                    for n in range(NCH):
                        ps = psA.tile([P, NN], f32, tag="psA")
                        for k in range(KC // 2):
                            nc.tensor.matmul(
                                ps[:],
                                mt_sb[:, 2 * k : 2 * k + 2, mc * P : (mc + 1) * P],
                                x8_sb[:, 2 * k : 2 * k + 2, n * NN : (n + 1) * NN],
                                start=(k == 0),
                                stop=(k == KC // 2 - 1),
                                perf_mode=DR,
                            )
                        nc.any.tensor_copy(
                            out=y_sb[:, mc, n * NN : (n + 1) * NN], in_=ps[:]
                        )

                # ---- stage A2b: vs broadcast via ones-matmul ----
                vsb = vpool.tile([P, S], f32, tag="vsb")
                for n in range(NCH):
                    psv = psA.tile([P, NN], f32, tag="psA")
                    nc.tensor.matmul(
                        psv[:], ones_sb[:],
                        vacc[:, n * NN : (n + 1) * NN],
                        start=True, stop=True,
                    )
                    nc.any.tensor_copy(out=vsb[:, n * NN : (n + 1) * NN], in_=psv[:])

                if b == 0:
                    # proj weights not needed until stage C; loading them here
                    # keeps the critical-path DMAs (mt, x8, x) uncontended.
                    for oc in range(KC):
                        nc.sync.dma_start(
                            wpt_sb[:, :, oc * P : (oc + 1) * P],
                            wpt_d[oc].rearrange("(ko ki) m -> ki ko m", ki=P),
                        )

                # ---- stage B: logits in f32r, fused softmax * vs -> h bf16 ----
                h_sb = hpool.tile([P, TC, S], bf16, tag="h")
                for tt in range(TC):
                    e = h_sb[:, tt, :]
                    rsh = []
                    for n in range(NCH):
                        psl = psB.tile([P, NN], f32, tag="psB")
                        for k in range(KC):
                            nc.tensor.matmul(
                                psl[:],
                                xc[k][:, tt * P : (tt + 1) * P],
                                y_sb[:, k, n * NN : (n + 1) * NN],
                                start=(k == 0),
                                stop=(k == KC - 1),
                            )
                        # e-half = exp(logits / sqrt(C)); rs = partial row sum
                        rs = spool.tile([P, 1], f32, tag="rs")
                        nc.scalar.activation(
                            e[:, n * NN : (n + 1) * NN], psl[:],
                            mybir.ActivationFunctionType.Exp,
                            scale=float(SCALE), accum_out=rs[:],
                        )
                        rsh.append(rs)
                    rst = spool.tile([P, 1], f32, tag="rst")
                    nc.vector.tensor_tensor(
                        rst[:], rsh[0][:], rsh[1][:], mybir.AluOpType.add
                    )
                    rcp = spool.tile([P, 1], f32, tag="rcp")
                    nc.vector.reciprocal(rcp[:], rst[:])
                    # normalize rows (per-partition scalar) on ACT
                    nc.scalar.activation(
                        e[:], e[:], mybir.ActivationFunctionType.Copy,
                        scale=rcp[:],
                    )
                    # h = attn * vs  (vs broadcast over partitions via vsb)
                    nc.vector.tensor_tensor(
                        e[:], e[:], vsb[:], mybir.AluOpType.mult
                    )

                # ---- stage C: out = w_proj @ h + x + b (bf16 matmuls) ----
                # last batch: no stage-A/B work follows, so spread C's psum
                # tiles over every pool — deeper pipelining at the tail
                cpools = (
                    [(psC, "psC"), (psA, "psA"), (psB, "psB")]
                    if b == bpc - 1
                    else [(psC, "psC")]
                )
                for oc in range(KC):
                    for n in range(NCH):
                        cp, ctag = cpools[(oc * NCH + n) % len(cpools)]
                        pso = cp.tile([P, NN], f32, tag=ctag)
                        for tt in range(TC):
                            nc.tensor.matmul(
                                pso[:],
                                wpt_sb[:, tt, oc * P : (oc + 1) * P],
                                h_sb[:, tt, n * NN : (n + 1) * NN],
                                start=(tt == 0),
                                stop=(tt == TC - 1),
                            )
                        osb = opool.tile([P, NN], f32, tag="osb")
                        # copy+bias off PSUM, alternating engines so neither
                        # ACT nor DVE serializes the PSUM-bank release
                        if (oc + n) % 2 == 0:
                            nc.scalar.activation(
                                osb[:], pso[:],
                                mybir.ActivationFunctionType.Identity,
                                bias=bp_sb[:, oc : oc + 1],
                            )
                        else:
                            nc.vector.tensor_scalar(
                                osb[:], pso[:], bp_sb[:, oc : oc + 1], None,
                                mybir.AluOpType.add,
                            )
                        nc.vector.tensor_tensor(
                            osb[:], osb[:],
                            xc[oc][:, n * NN : (n + 1) * NN].bitcast(f32),
                            mybir.AluOpType.add,
                        )
                        nc.sync.dma_start(
                            out_d[b, oc * P : (oc + 1) * P, n * NN : (n + 1) * NN],
                            osb[:],
                        )
    nc.compile()
    return nc


def _to_fp8(a):
    return np.clip(
        np.ascontiguousarray(a.astype(np.float32)), -240.0, 240.0
    ).astype(ml_dtypes.float8_e4m3fn)


def _round_f32r(a):
    """Round fp32 to float32r (11-bit mantissa, round-to-nearest-even)."""
    u = np.ascontiguousarray(a.astype(np.float32)).view(np.uint32)
    lsb = (u >> np.uint32(12)) & np.uint32(1)
    r = (u + np.uint32(0x7FF) + lsb) & np.uint32(0xFFFFF000)
    return r.view(np.float32)


def _host_prep(w_qkv, w_proj, b_proj):
    wq = w_qkv[0:C].astype(np.float64)
    wk = w_qkv[C : 2 * C].astype(np.float64)
    wv = w_qkv[2 * C : 3 * C]
    # lhsT for y-matmul: MT[c, c'] = M[c', c],  M = Wq^T Wk  =>  MT = Wk^T Wq
    mt = np.ascontiguousarray(wk.T @ wq).astype(np.float32)
    wvs = wv.sum(axis=0, dtype=np.float64).astype(np.float32)
    wpt = np.ascontiguousarray(w_proj.T).astype(np.float32)
    # stripe layout [chunk, c, 128]
    mt_s = np.ascontiguousarray(mt.reshape(C, KC, P).transpose(1, 0, 2))
    wpt_s = np.ascontiguousarray(wpt.reshape(C, KC, P).transpose(1, 0, 2))
    return (
        _to_fp8(mt_s),
        np.ascontiguousarray(wpt_s).astype(ml_dtypes.bfloat16),
        wvs,
        b_proj.astype(np.float32),
    )


_NC_CACHE = {}


def _get_nc(bpc=BPC):
    if bpc not in _NC_CACHE:
        _NC_CACHE[bpc] = build_nc(bpc)
    return _NC_CACHE[bpc]


def kernel(x, w_qkv, w_proj, b_proj, _trace=False):
    x = np.asarray(x, dtype=np.float32)
    mt, wpt, wvs, bp = _host_prep(
        np.asarray(w_qkv, np.float32),
        np.asarray(w_proj, np.float32),
        np.asarray(b_proj, np.float32),
    )
    xr_full = _round_f32r(x.reshape(B, C, S))
    # fp8 copy in [b, p, k, s] layout (c = k*128 + p)
    x8_full = _to_fp8(x.reshape(B, C, S)).reshape(B, KC, P, S).transpose(0, 2, 1, 3)
    ones = np.ones((P, P), np.float32)
    in_maps = []
    for c in range(N_CORES):
        sl = slice(c * BPC, (c + 1) * BPC)
        in_maps.append(
            {
                "x": np.ascontiguousarray(xr_full[sl]),
                "x8": np.ascontiguousarray(x8_full[sl]),
                "mt": mt,
                "wpt": wpt,
                "wvs": wvs,
                "ones": ones,
                "bp": bp,
            }
        )
    nc = _get_nc(BPC)
    res = run_bass_kernel_spmd(
        nc, in_maps, core_ids=list(range(N_CORES)), trace=_trace
    )
    out = np.concatenate([r["out"] for r in res.results], axis=0)
    out = out.reshape(B, C, HH, WW)
    if _trace:
        kernel.last_results = res
    return out


# revision 4
# speedup vs baseline: 1.0244x; 1.0244x over previous
"""Trainium2 Bass kernel for nn_AttentionBlock (B=32, C=1024, H=W=32, nh=1).

Reference computation (per batch b, with S = H*W = 1024):
    qkv = w_qkv @ x_b            # [3C, S], 1x1 conv == channel matmul
    q, k, v = split(qkv)
    logits[t,s] = (q[:,t] . k[:,s]) / sqrt(C)
    attn = softmax_s(logits)
    h[t,s] = attn[t,s] * sum_c v[c,s]
    out = w_proj @ h + b_proj + x_b

Algebraic simplifications (all weight-only, precomputed on host):
  * logits = x^T (M x) with M = Wq^T Wk  -> q/k never materialized.
  * sum_c v[c,s] = (sum_c Wv[c,:]) . x[:,s] = vs, computed on the
    vector/scalar engines + one ones-matmul (partition reduction).
  * h = attn .* (1 vs^T).

Precision plan (measured: fp8 DoubleRow = 1.87x bf16 matmul throughput):
  * stage A (y = M x): fp8e4 DoubleRow pairs, M and x quantized on host
    (clip to +-240 for TRN e4m3). Error here is softened by the softmax
    normalization downstream.
  * stage B (logits = x^T y): f32r x f32r (11+ mantissa bits) - the
    softmax input stays near-exact.
  * stage C (out = w_proj @ h): bf16 x bf16.
  * residual adds the resident f32 x exactly.
  Simulated end-to-end rel err ~9e-3 on both observed input draws
  (gate 2e-2).

Sharding: data-parallel over batch, 4 batches per core on 8 cores.
"""

import os
import sys

import numpy as np

for _p in ("/opt/trn_rl_repo", "/opt/pypackages"):
    if _p not in sys.path:
        sys.path.insert(0, _p)

import ml_dtypes

import concourse.bass as bass
import concourse.tile as tile
from concourse import bacc, mybir
from concourse.bass_utils import run_bass_kernel_spmd
from concourse.tile_rust import add_dep_helper

B, C, HH, WW = 32, 1024, 32, 32
S = HH * WW          # 1024 spatial positions
P = 128              # partitions
KC = C // P          # 8 chunks along channel dim
TC = S // P          # 8 chunks along spatial (t) dim
NN = 512             # matmul moving free dim
NCH = S // NN        # 2 free-dim halves
N_CORES = 8
BPC = B // N_CORES   # batches per core
SCALE = 1.0 / np.sqrt(float(C))  # folded into the exp

f32 = mybir.dt.float32
f32r = mybir.dt.float32r
bf16 = mybir.dt.bfloat16
fp8 = mybir.dt.float8e4
DR = mybir.MatmulPerfMode.DoubleRow


def build_nc(bpc: int = BPC):
    nc = bacc.Bacc(
        "TRN2",
        target_bir_lowering=False,
        debug=False,
        enable_asserts=False,
    )

    # x pre-rounded to f32r on host (stage-B stationary, vs input, and
    # the residual via bitcast back to f32 — costs ~2^-12 rounding)
    x_d = nc.dram_tensor("x", [bpc, C, S], f32r, kind="ExternalInput")
    # x pre-quantized to fp8 on host, layout [p, k, s] with c = k*128+p
    x8_d = nc.dram_tensor("x8", [bpc, P, KC, S], fp8, kind="ExternalInput")
    # weights pre-arranged on host in final SBUF layout (contiguous,
    # big DMA lines): [p, k, c'] with row c = k*128+p
    mt_d = nc.dram_tensor("mt", [P, KC, C], fp8, kind="ExternalInput")
    wpt_d = nc.dram_tensor("wpt", [P, TC, C], bf16, kind="ExternalInput")
    wvs_d = nc.dram_tensor("wvs", [P, KC], f32, kind="ExternalInput")
    ones_d = nc.dram_tensor("ones", [P, P], f32r, kind="ExternalInput")
    bp_d = nc.dram_tensor("bp", [P, KC], f32, kind="ExternalInput")
    out_d = nc.dram_tensor("out", [bpc, C, S], f32, kind="ExternalOutput")

    with tile.TileContext(nc) as tc:
        with (
            tc.tile_pool(name="weights", bufs=1) as wpool,
            tc.tile_pool(name="xc", bufs=16) as xpool,
            tc.tile_pool(name="x8", bufs=2) as x8pool,
            tc.tile_pool(name="y", bufs=1) as ypool,
            tc.tile_pool(name="h", bufs=1) as hpool,
            tc.tile_pool(name="vsb", bufs=2) as vpool,
            tc.tile_pool(name="vacc", bufs=2) as vaccpool,
            tc.tile_pool(name="vtmp", bufs=4) as vtmppool,
            tc.tile_pool(name="osb", bufs=4) as opool,
            tc.tile_pool(name="small", bufs=8) as spool,
            tc.tile_pool(name="psA", bufs=3, space="PSUM") as psA,
            tc.tile_pool(name="psB", bufs=2, space="PSUM") as psB,
            tc.tile_pool(name="psC", bufs=3, space="PSUM") as psC,
        ):
            # ---- ones first: the warmup matmuls depend only on it ----
            ones_sb = wpool.tile([P, P], f32r, tag="ones")
            nc.sync.dma_start(ones_sb[:], ones_d[:, :])
            wvs_sb = wpool.tile([P, KC], f32, tag="wvs")
            nc.sync.dma_start(wvs_sb[:], wvs_d[:, :])
            bp_sb = wpool.tile([P, KC], f32, tag="bp")
            nc.sync.dma_start(bp_sb[:], bp_d[:, :])
            # warm the PE clock (HAM) with throwaway matmuls while the
            # first batch's weights/x DMAs are in flight
            wu = psA.tile([P, NN], f32, tag="psA")
            for _ in range(25):
                nc.tensor.matmul(
                    wu[:, 0:64], ones_sb[:], ones_sb[:, 0:64],
                    start=True, stop=True,
                )
            wpt_sb = wpool.tile([P, TC, C], bf16, tag="wpt")
            mt_sb = wpool.tile([P, KC, C], fp8, tag="mt")

            for b in range(bpc):
                # ---- load x: fp8 (stage A) first, f32 chunks after ----
                x8_sb = x8pool.tile([P, KC, S], fp8, tag="x8")
                xc = []
                for k in range(KC):
                    t = xpool.tile([P, S], f32r, tag="xc")
                    xc.append(t)
                if b == 0:
                    # Critical startup set: mt (1 MiB fp8) + x8 (1 MiB) — the
                    # stage-A inputs, one contiguous DMA each. xc chains
                    # behind them (stage B needs it ~16us later), wpt after
                    # xc (stage C needs it ~60us later).
                    gate = nc.sync.dma_start(mt_sb[:], mt_d[:, :, :]).ins
                    nc.sync.dma_start(x8_sb[:], x8_d[b])
                    noncrit = []
                    for k in range(KC):
                        noncrit.append(
                            nc.sync.dma_start(xc[k][:], x_d[b, k * P : (k + 1) * P, :])
                        )
                    for inst in noncrit:
                        add_dep_helper(
                            inst.ins, gate, sync=True,
                            reason="startup: critical DMAs first",
                        )
                    wp_dma = nc.sync.dma_start(wpt_sb[:], wpt_d[:, :, :])
                    add_dep_helper(
                        wp_dma.ins, noncrit[-1].ins, sync=True,
                        reason="startup: xc before wpt",
                    )
                else:
                    nc.sync.dma_start(x8_sb[:], x8_d[b])
                    for k in range(KC):
                        nc.sync.dma_start(xc[k][:], x_d[b, k * P : (k + 1) * P, :])

                # ---- stage A2a: vacc[p,s] = sum_k wvs[k*128+p] * x[k][p,s]
                # products on ACT; pairwise-tree adds split over DVE and
                # GPSIMD so neither serializes. PE only does the final
                # 128-partition ones-matmul.
                vacc = vaccpool.tile([P, S], f32r, tag="vacc")

                def _vprod(k):
                    vt = vtmppool.tile([P, S], f32, tag="vtmp")
                    nc.scalar.activation(
                        vt[:], xc[k][:],
                        mybir.ActivationFunctionType.Copy,
                        scale=wvs_sb[:, k : k + 1],
                    )
                    return vt

                p0, p1 = _vprod(0), _vprod(1)
                nc.vector.tensor_tensor(p0[:], p0[:], p1[:], mybir.AluOpType.add)
                p2, p3 = _vprod(2), _vprod(3)
                nc.gpsimd.tensor_tensor(p2[:], p2[:], p3[:], mybir.AluOpType.add)
                nc.gpsimd.tensor_tensor(p0[:], p0[:], p2[:], mybir.AluOpType.add)
                p4, p5 = _vprod(4), _vprod(5)
                nc.vector.tensor_tensor(p4[:], p4[:], p5[:], mybir.AluOpType.add)
                p6, p7 = _vprod(6), _vprod(7)
                nc.gpsimd.tensor_tensor(p6[:], p6[:], p7[:], mybir.AluOpType.add)
                nc.vector.tensor_tensor(p4[:], p4[:], p6[:], mybir.AluOpType.add)
                nc.vector.tensor_tensor(vacc[:], p0[:], p4[:], mybir.AluOpType.add)

                # ---- stage A: y = M x via fp8 DoubleRow pairs ----
                y_sb = ypool.tile([P, KC, S], f32r, tag="y")
                for mc in range(KC):
                    for n in range(NCH):
                        ps = psA.tile([P, NN], f32, tag="psA")
                        for k in range(KC // 2):
                            nc.tensor.matmul(
                                ps[:],
                                mt_sb[:, 2 * k : 2 * k + 2, mc * P : (mc + 1) * P],
                                x8_sb[:, 2 * k : 2 * k + 2, n * NN : (n + 1) * NN],
                                start=(k == 0),
                                stop=(k == KC // 2 - 1),
                                perf_mode=DR,
                            )
                        nc.any.tensor_copy(
                            out=y_sb[:, mc, n * NN : (n + 1) * NN], in_=ps[:]
                        )

                # ---- stage A2b: vs broadcast via ones-matmul ----
                vsb = vpool.tile([P, S], f32, tag="vsb")
                for n in range(NCH):
                    psv = psA.tile([P, NN], f32, tag="psA")
                    nc.tensor.matmul(
                        psv[:], ones_sb[:],
                        vacc[:, n * NN : (n + 1) * NN],
                        start=True, stop=True,
                    )
                    nc.any.tensor_copy(out=vsb[:, n * NN : (n + 1) * NN], in_=psv[:])

                # ---- stage B: logits in f32r, fused softmax * vs -> h bf16 ----
                h_sb = hpool.tile([P, TC, S], bf16, tag="h")
                for tt in range(TC):
                    e = h_sb[:, tt, :]
                    rsh = []
                    for n in range(NCH):
                        psl = psB.tile([P, NN], f32, tag="psB")
                        for k in range(KC):
                            nc.tensor.matmul(
                                psl[:],
                                xc[k][:, tt * P : (tt + 1) * P],
                                y_sb[:, k, n * NN : (n + 1) * NN],
                                start=(k == 0),
                                stop=(k == KC - 1),
                            )
                        # e-half = exp(logits / sqrt(C)); rs = partial row sum
                        rs = spool.tile([P, 1], f32, tag="rs")
                        nc.scalar.activation(
                            e[:, n * NN : (n + 1) * NN], psl[:],
                            mybir.ActivationFunctionType.Exp,
                            scale=float(SCALE), accum_out=rs[:],
                        )
                        rsh.append(rs)
                    rst = spool.tile([P, 1], f32, tag="rst")
                    nc.vector.tensor_tensor(
                        rst[:], rsh[0][:], rsh[1][:], mybir.AluOpType.add
                    )
                    rcp = spool.tile([P, 1], f32, tag="rcp")
                    nc.vector.reciprocal(rcp[:], rst[:])
                    # normalize rows (per-partition scalar) on ACT
                    nc.scalar.activation(
                        e[:], e[:], mybir.ActivationFunctionType.Copy,
                        scale=rcp[:],
                    )
                    # h = attn * vs  (vs broadcast over partitions via vsb)
                    nc.vector.tensor_tensor(
                        e[:], e[:], vsb[:], mybir.AluOpType.mult
                    )

                # ---- stage C: out = w_proj @ h + x + b (bf16 matmuls) ----
                # last batch: no stage-A/B work follows, so spread C's psum
                # tiles over every pool — deeper pipelining at the tail
                cpools = (
                    [(psC, "psC"), (psA, "psA"), (psB, "psB")]
                    if b == bpc - 1
                    else [(psC, "psC")]
                )
                for oc in range(KC):
                    for n in range(NCH):
                        cp, ctag = cpools[(oc * NCH + n) % len(cpools)]
                        pso = cp.tile([P, NN], f32, tag=ctag)
                        for tt in range(TC):
                            nc.tensor.matmul(
                                pso[:],
                                wpt_sb[:, tt, oc * P : (oc + 1) * P],
                                h_sb[:, tt, n * NN : (n + 1) * NN],
                                start=(tt == 0),
                                stop=(tt == TC - 1),
                            )
                        osb = opool.tile([P, NN], f32, tag="osb")
                        # copy+bias off PSUM, alternating engines so neither
                        # ACT nor DVE serializes the PSUM-bank release
                        if (oc + n) % 2 == 0:
                            nc.scalar.activation(
                                osb[:], pso[:],
                                mybir.ActivationFunctionType.Identity,
                                bias=bp_sb[:, oc : oc + 1],
                            )
                        else:
                            nc.vector.tensor_scalar(
                                osb[:], pso[:], bp_sb[:, oc : oc + 1], None,
                                mybir.AluOpType.add,
                            )
                        nc.vector.tensor_tensor(
                            osb[:], osb[:],
                            xc[oc][:, n * NN : (n + 1) * NN].bitcast(f32),
                            mybir.AluOpType.add,
                        )
                        nc.sync.dma_start(
                            out_d[b, oc * P : (oc + 1) * P, n * NN : (n + 1) * NN],
                            osb[:],
                        )
    nc.compile()
    return nc


def _to_fp8(a):
    return np.clip(
        np.ascontiguousarray(a.astype(np.float32)), -240.0, 240.0
    ).astype(ml_dtypes.float8_e4m3fn)


def _round_f32r(a):
    """Round fp32 to float32r (11-bit mantissa, round-to-nearest-even)."""
    u = np.ascontiguousarray(a.astype(np.float32)).view(np.uint32)
    lsb = (u >> np.uint32(12)) & np.uint32(1)
    r = (u + np.uint32(0x7FF) + lsb) & np.uint32(0xFFFFF000)
    return r.view(np.float32)


def _host_prep(w_qkv, w_proj, b_proj):
    wq = w_qkv[0:C].astype(np.float64)
    wk = w_qkv[C : 2 * C].astype(np.float64)
    wv = w_qkv[2 * C : 3 * C]
    # lhsT for y-matmul: MT[c, c'] = M[c', c],  M = Wq^T Wk  =>  MT = Wk^T Wq
    mt = np.ascontiguousarray(wk.T @ wq).astype(np.float32)
    wvs = wv.sum(axis=0, dtype=np.float64).astype(np.float32)
    wpt = np.ascontiguousarray(w_proj.T).astype(np.float32)
    # SBUF layout [p, k, c'] with row index c = k*128 + p
    mt_s = np.ascontiguousarray(mt.reshape(KC, P, C).transpose(1, 0, 2))
    wpt_s = np.ascontiguousarray(wpt.reshape(TC, P, C).transpose(1, 0, 2))
    return (
        _to_fp8(mt_s),
        np.ascontiguousarray(wpt_s).astype(ml_dtypes.bfloat16),
        np.ascontiguousarray(wvs.reshape(KC, P).T),
        np.ascontiguousarray(b_proj.astype(np.float32).reshape(KC, P).T),
    )


_NC_CACHE = {}


def _get_nc(bpc=BPC):
    if bpc not in _NC_CACHE:
        _NC_CACHE[bpc] = build_nc(bpc)
    return _NC_CACHE[bpc]


def kernel(x, w_qkv, w_proj, b_proj, _trace=False):
    x = np.asarray(x, dtype=np.float32)
    mt, wpt, wvs, bp = _host_prep(
        np.asarray(w_qkv, np.float32),
        np.asarray(w_proj, np.float32),
        np.asarray(b_proj, np.float32),
    )
    xr_full = _round_f32r(x.reshape(B, C, S))
    # fp8 copy in [b, p, k, s] layout (c = k*128 + p)
    x8_full = _to_fp8(x.reshape(B, C, S)).reshape(B, KC, P, S).transpose(0, 2, 1, 3)
    ones = np.ones((P, P), np.float32)
    in_maps = []
    for c in range(N_CORES):
        sl = slice(c * BPC, (c + 1) * BPC)
        in_maps.append(
            {
                "x": np.ascontiguousarray(xr_full[sl]),
                "x8": np.ascontiguousarray(x8_full[sl]),
                "mt": mt,
                "wpt": wpt,
                "wvs": wvs,
                "ones": ones,
                "bp": bp,
            }
        )
    nc = _get_nc(BPC)
    res = run_bass_kernel_spmd(
        nc, in_maps, core_ids=list(range(N_CORES)), trace=_trace
    )
    out = np.concatenate([r["out"] for r in res.results], axis=0)
    out = out.reshape(B, C, HH, WW)
    if _trace:
        kernel.last_results = res
    return out
